# revision 6
# baseline (speedup 1.0000x reference)
"""Trainium2 Bass kernel for nn_DALayer (moe_routing, squeeze-excite style).

Computation (per sample b):
    y    = mean(x[b], axis=(H,W))                 # [C]
    h    = relu(W1[dataset[b]] @ y)               # [HID]
    gate = sigmoid(W2[dataset[b]] @ h)            # [C]
    out[b] = x[b] * gate[:, None, None]

Sharding: pure data parallel over batch across 8 NeuronCores (8 samples
per core); expert weights replicated.

The layer is HBM-bound: the only bulk traffic is streaming x in and out.
x is cast to fp16 on the host, so the device reads 32 MiB and writes
32 MiB per core instead of 64+64 in fp32 — the HBM-per-core roofline
(~358 GB/s) then gives a ~187 us floor instead of ~375 us.  The fp16
round-trip costs ~4e-4 relative L2 error, far inside the 2e-2 gate; all
small tensors (sums, expert MLP, gate) stay fp32.

Per sample, one 4 MiB DMA loads all of x[b] into a single SBUF tile
[128p, (j hw)] (channel chunk j = channels [128j, 128j+128)); the
channel sums come from an in-place identity tensor_scalar with
accum_out, which runs in the DVE's packed 4x mode (fp16, stride 1)
instead of tensor_reduce's 1x mode; the gate multiply is also a 4x-mode
tensor_scalar; one 4 MiB DMA stores the result.  Loads are issued on
the sync HWDGE ring and stores on the scalar ring so the two streams
flow on independent queues.

Expert routing is done on-device: all three experts' W1 rows are stacked
([96, C]) so one accumulating matmul chain produces h for every expert;
a mask (dataset broadcast across partitions + per-expert-block is_equal)
zeroes the two unselected experts' h, and a stacked-W2 matmul then
yields the selected expert's gate directly.
"""

import os

import numpy as np
from contextlib import ExitStack

import concourse.tile as tile
from concourse import bacc, mybir
from concourse import bass_utils

# Problem shapes (hardcoded per contract).
B, C, H, W = 64, 512, 64, 64
HW = H * W                 # 4096 spatial elements
N_CORES = 8
BL = B // N_CORES          # 8 samples per core
NE, HID = 3, 32
M96 = NE * HID             # 96 stacked expert-hidden rows
P = 128                    # SBUF partitions
J = C // P                 # 4 channel chunks of 128
FREE = J * HW              # 16384 free elements per partition per sample

_nc_cache = {}


def _build(passes=1):
    """Build + compile the per-core Bass module (cached).

    passes>1 repeats the whole pipeline (for timing: T(2)-T(1) cancels
    fixed dispatch overhead)."""
    if passes in _nc_cache:
        return _nc_cache[passes]

    f16 = mybir.dt.float16
    f32 = mybir.dt.float32
    i32 = mybir.dt.int32
    FT = mybir.ActivationFunctionType

    nc = bacc.Bacc(
        "TRN2",
        target_bir_lowering=False,
        debug=False,
        enable_asserts=False,
        num_devices=N_CORES,
    )
    x = nc.dram_tensor("x", [BL, C, H, W], f16, kind="ExternalInput").ap()
    d = nc.dram_tensor("d", [1, BL], i32, kind="ExternalInput").ap()
    w1t = nc.dram_tensor("w1t", [C, M96], f32, kind="ExternalInput").ap()
    w2t = nc.dram_tensor("w2t", [M96, C], f32, kind="ExternalInput").ap()
    out = nc.dram_tensor("out", [BL, C, H, W], f16, kind="ExternalOutput").ap()

    # [b, c, (h w)] views; per-sample DMAs below re-split c into (j p)
    # with chunk j = channels [128j, 128j+128), matching SBUF tile
    # [p, (j hw)] layouts.
    xr = x.rearrange("b c h w -> b c (h w)")
    outr = out.rearrange("b c h w -> b c (h w)")

    with ExitStack() as ctx:
        tc = ctx.enter_context(tile.TileContext(nc))
        const = ctx.enter_context(tc.tile_pool(name="const", bufs=1))
        xpool = ctx.enter_context(tc.tile_pool(name="xp", bufs=4))
        small = ctx.enter_context(tc.tile_pool(name="small", bufs=4))
        ps_h = ctx.enter_context(tc.tile_pool(name="psh", bufs=2, space="PSUM"))
        ps_g = ctx.enter_context(tc.tile_pool(name="psg", bufs=2, space="PSUM"))

        # ---- weights / routing constants (tiny, loaded once) ----
        # w1_sb columns [96j, 96j+96) hold chunk j: lhsT [K=128 c, M=96 (e,hid)]
        w1_sb = const.tile([P, J * M96], f32)
        for j in range(J):
            nc.sync.dma_start(w1_sb[:, j * M96:(j + 1) * M96], w1t[j * P:(j + 1) * P, :])
        w2_sb = const.tile([M96, C], f32)       # lhsT [K=96, M=128] per c-chunk
        nc.sync.dma_start(w2_sb[:], w2t)
        # dataset replicated across 96 partitions (stride-0 DMA read), cast,
        # then mask[32e+k, b] = (dataset[b] == e) built per 32-aligned block
        di_bc = const.tile([M96, BL], i32)
        nc.sync.dma_start(di_bc[:], d.broadcast_to([M96, BL]))
        df_bc = const.tile([M96, BL], f32)
        nc.vector.tensor_copy(df_bc[:], di_bc[:])          # int32 -> f32 cast
        m_sb = const.tile([M96, BL], f32)
        for e in range(NE):
            nc.vector.tensor_scalar(
                m_sb[e * HID:(e + 1) * HID, :], df_bc[e * HID:(e + 1) * HID, :],
                float(e), None, op0=mybir.AluOpType.is_equal,
            )

        # ---- per-sample pipeline ----
        for b in [bb for _ in range(passes) for bb in range(BL)]:
            xt = xpool.tile([P, FREE], f16, tag="xt")
            src = xr[b].rearrange("(j p) s -> p j s", p=P)
            dst = xt[:].rearrange("p (j s) -> p j s", j=J)
            nc.sync.dma_start(dst, src)
            # channel sums (mean * HW, scale folded into the relu below) via
            # in-place identity tensor_scalar + accum_out: packed 4x DVE mode
            ysum = small.tile([P, J], f32, tag="y")
            for j in range(J):
                sl = xt[:, j * HW:(j + 1) * HW]
                nc.vector.tensor_scalar(
                    sl, sl, 1.0, 0.0, op0=mybir.AluOpType.mult,
                    op1=mybir.AluOpType.add, accum_out=ysum[:, j:j + 1],
                )
            # h for all 3 experts at once: [96, 1]
            h_ps = ps_h.tile([M96, 1], f32, tag="h")
            for j in range(J):
                nc.tensor.matmul(
                    h_ps[:], w1_sb[:, j * M96:(j + 1) * M96], ysum[:, j:j + 1],
                    start=(j == 0), stop=(j == J - 1),
                )
            h_sb = small.tile([M96, 1], f32, tag="hs")
            nc.scalar.activation(h_sb[:], h_ps[:], FT.Relu, scale=1.0 / HW)
            hm_sb = small.tile([M96, 1], f32, tag="hm")
            nc.vector.tensor_mul(hm_sb[:], h_sb[:], m_sb[:, b:b + 1])
            # gate[c] for the selected expert, c-chunk j in column j
            g_ps = ps_g.tile([P, J], f32, tag="g")
            for j in range(J):
                nc.tensor.matmul(
                    g_ps[:, j:j + 1], w2_sb[:, j * P:(j + 1) * P], hm_sb[:],
                    start=True, stop=True,
                )
            g_sb = small.tile([P, J], f32, tag="gs")
            nc.scalar.activation(g_sb[:], g_ps[:], FT.Sigmoid)
            # apply gate in place (4x-mode tensor_scalar, per-partition f32
            # scalar), then store the whole sample with one DMA on the
            # scalar engine's HWDGE ring
            for j in range(J):
                sl = xt[:, j * HW:(j + 1) * HW]
                nc.vector.tensor_scalar(
                    sl, sl, g_sb[:, j:j + 1], None, op0=mybir.AluOpType.mult,
                )
            odst = outr[b].rearrange("(j p) s -> p j s", p=P)
            osrc = xt[:].rearrange("p (j s) -> p j s", j=J)
            nc.scalar.dma_start(odst, osrc)

    nc.compile()
    _nc_cache[passes] = nc
    return nc


def _prep_shared(W1, W2):
    # lhsT layouts: w1t[c, 32e+k] = W1[e, k, c]; w2t[32e+k, c] = W2[e, c, k]
    w1t = np.ascontiguousarray(W1.transpose(2, 0, 1).reshape(C, M96)).astype(np.float32, copy=False)
    w2t = np.ascontiguousarray(W2.transpose(0, 2, 1).reshape(M96, C)).astype(np.float32, copy=False)
    return w1t, w2t


def _prep_in_maps(x, dataset, W1, W2):
    x16 = np.asarray(x).astype(np.float16)
    w1t, w2t = _prep_shared(np.asarray(W1), np.asarray(W2))
    dataset = np.asarray(dataset, dtype=np.int32)
    in_maps = []
    for c in range(N_CORES):
        sl = slice(c * BL, (c + 1) * BL)
        in_maps.append({
            "x": x16[sl],
            "d": np.ascontiguousarray(dataset[sl].reshape(1, BL)),
            "w1t": w1t,
            "w2t": w2t,
        })
    return in_maps


def kernel(x, dataset, W1, W2):
    # NTFF tracing is unavailable under this axon client (antenv.axon_hooks
    # missing); make sure an inherited BASS_TRACE can't divert us into it.
    os.environ["BASS_NEVER_TRACE"] = "1"
    nc = _build()
    in_maps = _prep_in_maps(x, dataset, W1, W2)
    res = bass_utils.run_bass_kernel_spmd(
        nc, in_maps, core_ids=list(range(N_CORES)),
    )
    return np.concatenate([r["out"] for r in res.results], axis=0).astype(np.float32)
